# revision 4
# baseline (speedup 1.0000x reference)
"""Cross-view attention (nn_CrossViewAttention) Trainium2 Bass kernel.

Reference computation (B=2, N=4096, D=512):
    co    = relu(concat([x_f, x_s], -1) @ Wc.T + bc)
    out_f = attend(x_f@Wq.T+bq, x_s@Wk.T+bk, x_f@Wv.T+bv) + co
    out_s = attend(x_s@Wq.T+bq, x_f@Wk.T+bk, x_s@Wv.T+bv) + co
    attend(Q,K,V) = (softmax(Q K^T) / L1 / sqrt(D)) @ V

Sharding: 8 cores = (direction f/s) x (batch 0/1) x (sequence half).
Each core computes 2048 output rows of one direction against the full
4096-row K/V for its (direction, batch), SPMD with per-core input data.
Rows are permuted host-side so every core's own rows come first; the
attention reduction over keys is permutation invariant.  K, V and co
are computed fully locally -- collectives proved to serialize the
whole kernel for only a minor PE saving, so there are none.

Design notes (v2):
  - Q/K/V projections run in fp8-e4m3 DoubleRow (host ships e4m3 copies
    of x^T and the weights): half the PE time of the bf16 versions.
    The co MLP stays bf16 -- co dominates the output magnitude, fp8
    there fails the accuracy gate.  bf16 x is only shipped for the
    core's own 2048 rows (co's input).
  - Input DMAs are split across the two HWDGE queues (sync + scalar):
    the K-projection operands (wk8, xB8) go first on the sync queue so
    the first matmuls start ~2us in instead of ~30us.
  - Scores run e4m3 DoubleRow into [128,1024] PSUM tiles (2 banks),
    halving the ACT exp instruction count.  exp outputs bf16 tiles.
  - Row sums come from an all-ones [128,128] stationary matmul, so the
    sum lands in PSUM already broadcast across partitions; a single DVE
    reciprocal produces bf16 1/rowsum with no DRAM bounce, and the 32
    normalize multiplies run in the DVE 2x bf16 mode, emitting e4m3
    probabilities in [0,1].
  - PV runs e4m3 DoubleRow; the epilogue is one fused DVE op
    out = pv * (1/sqrt(D)) + co.
  - co's bc bias enters via a ones(x)bc matmul accumulated into the
    same PSUM group; relu and the +bv/sqrt(D) term fuse into one DVE
    scalar_tensor_tensor (max 0, add bv).
"""

import sys
from contextlib import ExitStack

for _p in ("/opt/trn_rl_repo", "/root/.axon_site/_ro/trn_rl_repo"):
    if _p not in sys.path:
        sys.path.insert(0, _p)

import ml_dtypes
import numpy as np

import concourse.bacc as bacc
import concourse.bass as bass
import concourse.mybir as mybir
import concourse.tile as tile

P = 128
D = 512
DC = D // P  # contraction chunks of 128
INV_SQRT_D = float(1.0 / np.sqrt(D))
EXP_SHIFT = -40.0

F32 = mybir.dt.float32
BF16 = mybir.dt.bfloat16
FP8 = mybir.dt.float8e4
AF = mybir.ActivationFunctionType
ALU = mybir.AluOpType
DR = mybir.MatmulPerfMode.DoubleRow


def build_program(nq, nkv, reps=1):
    nc = bacc.Bacc("TRN2", target_bir_lowering=False, debug=False, num_devices=8)

    # fp8 feature-major views (full rows, own query-half columns first)
    xA8 = nc.dram_tensor("xA8", [D, nkv], FP8, kind="ExternalInput").ap()
    xB8 = nc.dram_tensor("xB8", [D, nkv], FP8, kind="ExternalInput").ap()
    # bf16 feature-major views, own rows only (co MLP input)
    xATh = nc.dram_tensor("xATh", [D, nq], BF16, kind="ExternalInput").ap()
    xBTh = nc.dram_tensor("xBTh", [D, nq], BF16, kind="ExternalInput").ap()
    wq8 = nc.dram_tensor("wq8", [D, D], FP8, kind="ExternalInput").ap()
    wk8 = nc.dram_tensor("wk8", [D, D], FP8, kind="ExternalInput").ap()
    wv8 = nc.dram_tensor("wv8", [D, D], FP8, kind="ExternalInput").ap()
    wcAT = nc.dram_tensor("wcAT", [D, D], BF16, kind="ExternalInput").ap()
    wcBT = nc.dram_tensor("wcBT", [D, D], BF16, kind="ExternalInput").ap()
    bq_pc = nc.dram_tensor("bq_pc", [P, DC], F32, kind="ExternalInput").ap()
    bk_pc = nc.dram_tensor("bk_pc", [P, DC], F32, kind="ExternalInput").ap()
    bc_row = nc.dram_tensor("bc_row", [1, D], BF16, kind="ExternalInput").ap()
    bv_bc = nc.dram_tensor("bv_bc", [P, D], F32, kind="ExternalInput").ap()
    out = nc.dram_tensor("out", [nq, D], F32, kind="ExternalOutput").ap()

    with tile.TileContext(nc) as tc:
        for rep in range(reps):
            _emit_body(
                nc, tc, xA8, xB8, xATh, xBTh, wq8, wk8, wv8, wcAT, wcBT,
                bq_pc, bk_pc, bc_row, bv_bc, out, nq, nkv, rep,
            )

    nc.compile()
    return nc


def _emit_body(
    nc, tc, xA8, xB8, xATh, xBTh, wq8, wk8, wv8, wcAT, wcBT,
    bq_pc, bk_pc, bc_row, bv_bc, out, nq, nkv, rep,
):
    NBQ = nq // P   # query row blocks (16)
    MCK = nkv // P  # key row chunks (32)

    with ExitStack() as st:
        persist = st.enter_context(tc.tile_pool(name="persist", bufs=1))

        # ---------------- phase 1: projections (all local) ----------------
        with ExitStack() as ph1:
            xh_pool = ph1.enter_context(tc.tile_pool(name="xh", bufs=1))
            ps1 = ph1.enter_context(tc.tile_pool(name="ps1", bufs=4, space="PSUM"))

            # ---- critical-path loads on the sync HWDGE queue: K, then Q ----
            wk_sb = persist.tile([P, DC, D], FP8, name="w_wk")
            nc.sync.dma_start(out=wk_sb, in_=wk8.rearrange("(c p) o -> p c o", p=P))
            xB8_sb = xh_pool.tile([P, DC, nkv], FP8, name="xB8_sb")
            xB8_r = xB8.rearrange("(c p) n -> p c n", p=P)
            for n0 in range(0, nkv, 1024):
                nc.sync.dma_start(
                    out=xB8_sb[:, :, n0 : n0 + 1024], in_=xB8_r[:, :, n0 : n0 + 1024]
                )
            bk_sb = persist.tile([P, DC], F32, name="bk_sb")
            nc.sync.dma_start(out=bk_sb, in_=bk_pc)
            wq_sb = persist.tile([P, DC, D], FP8, name="w_wq")
            nc.sync.dma_start(out=wq_sb, in_=wq8.rearrange("(c p) o -> p c o", p=P))
            xA8_sb = xh_pool.tile([P, DC, nkv], FP8, name="xA8_sb")
            xA8_r = xA8.rearrange("(c p) n -> p c n", p=P)
            for n0 in range(0, nkv, 1024):
                nc.sync.dma_start(
                    out=xA8_sb[:, :, n0 : n0 + 1024], in_=xA8_r[:, :, n0 : n0 + 1024]
                )
            bq_sb = persist.tile([P, DC], F32, name="bq_sb")
            nc.sync.dma_start(out=bq_sb, in_=bq_pc)

            # ---- everything else on the scalar HWDGE queue ----
            wv_sb = persist.tile([P, DC, D], FP8, name="w_wv")
            nc.scalar.dma_start(out=wv_sb, in_=wv8.rearrange("(c p) o -> p c o", p=P))
            wcA_sb = persist.tile([P, DC, D], BF16, name="w_wcA")
            nc.scalar.dma_start(out=wcA_sb, in_=wcAT.rearrange("(c p) o -> p c o", p=P))
            wcB_sb = persist.tile([P, DC, D], BF16, name="w_wcB")
            nc.scalar.dma_start(out=wcB_sb, in_=wcBT.rearrange("(c p) o -> p c o", p=P))
            bc_sb = persist.tile([1, D], BF16, name="bc_sb")
            nc.scalar.dma_start(out=bc_sb, in_=bc_row)
            bv_sb = persist.tile([P, D], F32, name="bv_sb")
            nc.scalar.dma_start(out=bv_sb, in_=bv_bc)

            ones_row = persist.tile([1, P], BF16, name="ones_row")
            nc.vector.memset(ones_row, 1.0)
            ones_mat = persist.tile([P, P], BF16, name="ones_mat")
            nc.vector.memset(ones_mat, 1.0)
            shift_sb = persist.tile([P, 1], F32, name="shift_sb")
            nc.vector.memset(shift_sb, EXP_SHIFT)

            qT_sb = persist.tile([P, DC, nq], FP8, name="qT_sb")
            kT_sb = persist.tile([P, DC, nkv], FP8, name="kT_sb")
            v_sb = persist.tile([P, MCK, D], FP8, name="v_sb")
            co_sb = persist.tile([P, NBQ, D], BF16, name="co_sb")

            xAh_sb = xh_pool.tile([P, DC, nq], BF16, name="xAh_sb")
            xBh_sb = xh_pool.tile([P, DC, nq], BF16, name="xBh_sb")
            for src, dst in ((xATh, xAh_sb), (xBTh, xBh_sb)):
                src_r = src.rearrange("(c p) n -> p c n", p=P)
                for n0 in range(0, nq, 1024):
                    nc.scalar.dma_start(
                        out=dst[:, :, n0 : n0 + 1024],
                        in_=src_r[:, :, n0 : n0 + 1024],
                    )

            # K over all keys (fp8 DoubleRow), then Q over own rows
            for s0 in range(0, nkv, 512):
                for ob in range(DC):
                    ps = ps1.tile([P, 512], F32, name="ps_k", tag="ps1")
                    for c2 in range(DC // 2):
                        nc.tensor.matmul(
                            ps,
                            lhsT=wk_sb[:, 2 * c2 : 2 * c2 + 2, ob * P : (ob + 1) * P],
                            rhs=xB8_sb[:, 2 * c2 : 2 * c2 + 2, s0 : s0 + 512],
                            start=(c2 == 0),
                            stop=(c2 == DC // 2 - 1),
                            perf_mode=DR,
                        )
                    nc.scalar.activation(
                        out=kT_sb[:, ob, s0 : s0 + 512],
                        in_=ps,
                        func=AF.Identity,
                        bias=bk_sb[:, ob : ob + 1],
                        scale=1.0,
                    )
            for s0 in range(0, nq, 512):
                for ob in range(DC):
                    ps = ps1.tile([P, 512], F32, name="ps_q", tag="ps1")
                    for c2 in range(DC // 2):
                        nc.tensor.matmul(
                            ps,
                            lhsT=wq_sb[:, 2 * c2 : 2 * c2 + 2, ob * P : (ob + 1) * P],
                            rhs=xA8_sb[:, 2 * c2 : 2 * c2 + 2, s0 : s0 + 512],
                            start=(c2 == 0),
                            stop=(c2 == DC // 2 - 1),
                            perf_mode=DR,
                        )
                    nc.scalar.activation(
                        out=qT_sb[:, ob, s0 : s0 + 512],
                        in_=ps,
                        func=AF.Identity,
                        bias=bq_sb[:, ob : ob + 1],
                        scale=1.0,
                    )

            # co = relu(xA@WcA.T + xB@WcB.T + bc) + bv/sqrt(D), own rows.
            # bc enters the PSUM group via a ones(x)bc matmul; relu + bv
            # fuse into one DVE scalar_tensor_tensor.  bv is NOT part of
            # attention: rows sum to 1 after L1-normalize and the kernel
            # scales by 1/sqrt(D), so bv enters as +bv/sqrt(D) here.
            for nb in range(NBQ):
                ps = ps1.tile([P, 512], F32, name="ps_c", tag="ps1")
                nc.tensor.matmul(
                    ps, lhsT=ones_row, rhs=bc_sb, start=True, stop=False
                )
                for c in range(DC):
                    nc.tensor.matmul(
                        ps,
                        lhsT=xAh_sb[:, c, nb * P : (nb + 1) * P],
                        rhs=wcA_sb[:, c, :],
                        start=False,
                        stop=False,
                    )
                for c in range(DC):
                    nc.tensor.matmul(
                        ps,
                        lhsT=xBh_sb[:, c, nb * P : (nb + 1) * P],
                        rhs=wcB_sb[:, c, :],
                        start=False,
                        stop=(c == DC - 1),
                    )
                nc.vector.scalar_tensor_tensor(
                    out=co_sb[:, nb, :],
                    in0=ps,
                    scalar=0.0,
                    in1=bv_sb,
                    op0=ALU.max,
                    op1=ALU.add,
                )

            # V over all key rows of the A view (fp8 DoubleRow), e4m3 out
            for m in range(MCK):
                ps = ps1.tile([P, 512], F32, name="ps_v", tag="ps1")
                for c2 in range(DC // 2):
                    nc.tensor.matmul(
                        ps,
                        lhsT=xA8_sb[:, 2 * c2 : 2 * c2 + 2, m * P : (m + 1) * P],
                        rhs=wv_sb[:, 2 * c2 : 2 * c2 + 2, :],
                        start=(c2 == 0),
                        stop=(c2 == DC // 2 - 1),
                        perf_mode=DR,
                    )
                nc.scalar.activation(out=v_sb[:, m, :], in_=ps, func=AF.Copy)

        # ---------------- phase 2: attention (S^T layout) ----------------
        at_pool = st.enter_context(tc.tile_pool(name="at_pool", bufs=2))
        a8_pool = st.enter_context(tc.tile_pool(name="a8_pool", bufs=2))
        r_pool = st.enter_context(tc.tile_pool(name="r_pool", bufs=2))
        o_pool = st.enter_context(tc.tile_pool(name="o_pool", bufs=3))
        sps_pool = st.enter_context(tc.tile_pool(name="sps", bufs=2, space="PSUM"))
        sum_pool = st.enter_context(tc.tile_pool(name="sump", bufs=1, space="PSUM"))
        pv_pool = st.enter_context(tc.tile_pool(name="pv", bufs=3, space="PSUM"))

        for s0 in range(0, nq, 512):
            at_sb = at_pool.tile([P, MCK, 512], BF16, name="at_sb", tag="at")
            at8 = a8_pool.tile([P, MCK, 512], FP8, name="at8", tag="at8")
            # rowsum, broadcast across partitions by the all-ones lhsT
            ssum = sum_pool.tile([P, 512], F32, name="ssum", tag="ssum")
            for mbp in range(MCK // 2):
                sps = sps_pool.tile([P, 1024], F32, name="sps", tag="sps")
                for mi in range(2):
                    mb = 2 * mbp + mi
                    for c2 in range(DC // 2):
                        nc.tensor.matmul(
                            sps[:, mi * 512 : (mi + 1) * 512],
                            lhsT=kT_sb[:, 2 * c2 : 2 * c2 + 2, mb * P : (mb + 1) * P],
                            rhs=qT_sb[:, 2 * c2 : 2 * c2 + 2, s0 : s0 + 512],
                            start=(c2 == 0),
                            stop=(c2 == DC // 2 - 1),
                            perf_mode=DR,
                        )
                nc.scalar.activation(
                    out=at_sb[:, 2 * mbp : 2 * mbp + 2, :],
                    in_=sps,
                    func=AF.Exp,
                    bias=shift_sb,
                    scale=1.0,
                )
                for mi in range(2):
                    mb = 2 * mbp + mi
                    nc.tensor.matmul(
                        ssum,
                        lhsT=ones_mat,
                        rhs=at_sb[:, mb, :],
                        start=(mb == 0),
                        stop=(mb == MCK - 1),
                    )
            # 1/rowsum, already broadcast across partitions; bf16 so the
            # normalize multiplies run in the DVE 2x packed mode
            rs_bc = r_pool.tile([P, 512], BF16, name="rs_bc", tag="rsb")
            with nc.allow_low_precision(reason="bf16 1/rowsum: 0.4% on a term ~5% of |out|"):
                nc.vector.reciprocal(out=rs_bc, in_=ssum)
            for mb in range(MCK):
                nc.vector.tensor_mul(at8[:, mb, :], at_sb[:, mb, :], rs_bc)

            for j in range(4):
                pv = pv_pool.tile([P, D], F32, name="pv", tag="pv")
                for i2 in range(MCK // 2):
                    nc.tensor.matmul(
                        pv,
                        lhsT=at8[:, 2 * i2 : 2 * i2 + 2, j * P : (j + 1) * P],
                        rhs=v_sb[:, 2 * i2 : 2 * i2 + 2, :],
                        start=(i2 == 0),
                        stop=(i2 == MCK // 2 - 1),
                        perf_mode=DR,
                    )
                nb = s0 // P + j
                outt = o_pool.tile([P, D], F32, name="outt", tag="outt")
                nc.vector.scalar_tensor_tensor(
                    out=outt,
                    in0=pv,
                    scalar=INV_SQRT_D,
                    in1=co_sb[:, nb, :],
                    op0=ALU.mult,
                    op1=ALU.add,
                )
                nc.sync.dma_start(out=out[nb * P : (nb + 1) * P, :], in_=outt)


_PROG_CACHE = {}


def _get_program(nq, nkv):
    key = (nq, nkv)
    if key not in _PROG_CACHE:
        _PROG_CACHE[key] = build_program(nq, nkv)
    return _PROG_CACHE[key]


def make_in_maps(x_f, x_s, Wq, bq, Wk, bk, Wv, bv, Wc, bc):
    """Per-core SPMD input dicts + (direction, batch, half) layout.

    x ships feature-major: e4m3 full 4096 rows (Q/K/V projections) and
    bf16 own-half rows (co MLP), with the core's own query-half rows
    first.  Weights ship transposed: e4m3 for Wq/Wk/Wv, bf16 for Wc.
    """
    x_f = np.asarray(x_f, np.float32)
    x_s = np.asarray(x_s, np.float32)
    B, N, _ = x_f.shape
    nq = N // 2
    bf = ml_dtypes.bfloat16
    e4 = ml_dtypes.float8_e4m3
    Wq8 = np.ascontiguousarray(np.asarray(Wq, np.float32).T).astype(e4)
    Wk8 = np.ascontiguousarray(np.asarray(Wk, np.float32).T).astype(e4)
    Wv8 = np.ascontiguousarray(np.asarray(Wv, np.float32).T).astype(e4)
    Wc = np.asarray(Wc, np.float32)
    WcfT = np.ascontiguousarray(Wc[:, :D].T).astype(bf)
    WcsT = np.ascontiguousarray(Wc[:, D:].T).astype(bf)
    bq32, bk32, bv32, bc32 = (
        np.asarray(b, np.float32) for b in (bq, bk, bv, bc)
    )
    bq_pc = np.ascontiguousarray(bq32.reshape(DC, P).T)
    bk_pc = np.ascontiguousarray(bk32.reshape(DC, P).T)
    bc_row = np.ascontiguousarray(bc32[None, :]).astype(bf)
    # Attention rows sum to 1 after L1 normalization and the kernel scales
    # by 1/sqrt(D), so bv enters the output as bv/sqrt(D), added via co.
    bv_bc = np.ascontiguousarray(
        np.broadcast_to(bv32 / np.sqrt(D, dtype=np.float32), (P, D))
    )
    xT_f = [np.ascontiguousarray(x_f[b].T) for b in range(B)]
    xT_s = [np.ascontiguousarray(x_s[b].T) for b in range(B)]
    in_maps, layout = [], []
    for d in range(2):
        for b in range(B):
            for h in range(2):
                xq = xT_f[b] if d == 0 else xT_s[b]
                xk = xT_s[b] if d == 0 else xT_f[b]
                if h == 1:
                    idx = np.r_[nq:N, 0:nq]
                    xq, xk = xq[:, idx], xk[:, idx]
                in_maps.append(
                    {
                        "xA8": np.ascontiguousarray(xq).astype(e4),
                        "xB8": np.ascontiguousarray(xk).astype(e4),
                        "xATh": np.ascontiguousarray(xq[:, :nq]).astype(bf),
                        "xBTh": np.ascontiguousarray(xk[:, :nq]).astype(bf),
                        "wq8": Wq8,
                        "wk8": Wk8,
                        "wv8": Wv8,
                        "wcAT": WcfT if d == 0 else WcsT,
                        "wcBT": WcsT if d == 0 else WcfT,
                        "bq_pc": bq_pc,
                        "bk_pc": bk_pc,
                        "bc_row": bc_row,
                        "bv_bc": bv_bc,
                    }
                )
                layout.append((d, b, h))
    return in_maps, layout


def kernel(x_f, x_s, Wq, bq, Wk, bk, Wv, bv, Wc, bc):
    x_f = np.asarray(x_f, np.float32)
    B, N, _ = x_f.shape
    nq = N // 2
    nc = _get_program(nq, N)
    in_maps, layout = make_in_maps(x_f, x_s, Wq, bq, Wk, bk, Wv, bv, Wc, bc)

    from concourse.bass_utils import run_bass_kernel_spmd

    res = run_bass_kernel_spmd(nc, in_maps, list(range(len(in_maps))))
    out_f = np.empty((B, N, D), np.float32)
    out_s = np.empty((B, N, D), np.float32)
    for (d, b, h), r in zip(layout, res.results):
        tgt = out_f if d == 0 else out_s
        tgt[b, h * nq : (h + 1) * nq] = r["out"]
    return out_f, out_s


# revision 10
# speedup vs baseline: 1.0264x; 1.0264x over previous
"""Cross-view attention (nn_CrossViewAttention) Trainium2 Bass kernel.

Reference computation (B=2, N=4096, D=512):
    co    = relu(concat([x_f, x_s], -1) @ Wc.T + bc)
    out_f = attend(x_f@Wq.T+bq, x_s@Wk.T+bk, x_f@Wv.T+bv) + co
    out_s = attend(x_s@Wq.T+bq, x_f@Wk.T+bk, x_s@Wv.T+bv) + co
    attend(Q,K,V) = (softmax(Q K^T) / L1 / sqrt(D)) @ V

Sharding: 8 cores = (direction f/s) x (batch 0/1) x (sequence half).
Each core computes 2048 output rows of one direction against the full
4096-row K/V for its (direction, batch), SPMD with per-core input data.
Rows are permuted host-side so every core's own rows come first; the
attention reduction over keys is permutation invariant.  Collectives
proved to serialize the whole kernel for a minor PE saving, so K/V/co
are computed fully locally.

Design notes (v3):
  - Q/K/V projections run in fp8-e4m3 DoubleRow (host ships e4m3 copies
    of x^T and the weights); the co MLP stays bf16 (co dominates the
    output magnitude; fp8 there fails the accuracy gate).  bf16 x ships
    only for the core's own 2048 rows (co's input).
  - Every DMA-able tensor is pre-arranged on the host so each partition
    reads one fat contiguous run (4-16 KiB descriptors); the v2 layout
    produced 0.5-1 KiB descriptors that starved the first matmuls.
  - Input DMAs split across the two HWDGE queues (sync carries the K/Q
    operands, scalar everything else) so the first matmuls start right
    after the NEFF preamble.
  - Projections accumulate into [128,2048] PSUM tiles (4 banks, one
    8-bank pool of 2) and drain with FD=2048 ACT ops, so ACT no longer
    paces the PE during phase 1 (v2 lost ~146ns per 2-matmul group).
  - co and PV produce TRANSPOSED tiles (partitions = output feature):
    bc+relu fuse into the co ACT drain as a per-partition bias, bv
    enters the PV PSUM group via a ones-row matmul (rows sum to 1 after
    L1-normalize, so +bv there lands as +bv/sqrt(D) in the output), and
    the epilogue stays one fused DVE op out^T = pv^T/sqrt(D) + co^T.
    The kernel emits out^T [D, nq]; the host transposes for free.
  - Scores run e4m3 DoubleRow into [128,1024] PSUM tiles; exp outputs
    bf16 at FD=1024.  Row sums come from an all-ones [128,128]
    stationary matmul so they land in PSUM already broadcast across
    partitions; one DVE reciprocal yields bf16 1/rowsum, and the
    normalize multiplies run in the DVE 2x bf16 mode, emitting e4m3
    probabilities in [0,1] for the PV DoubleRow matmul.
"""

import sys
from contextlib import ExitStack

for _p in ("/opt/trn_rl_repo", "/root/.axon_site/_ro/trn_rl_repo"):
    if _p not in sys.path:
        sys.path.insert(0, _p)

import ml_dtypes
import numpy as np

import concourse.bacc as bacc
import concourse.bass as bass
import concourse.mybir as mybir
import concourse.tile as tile

P = 128
D = 512
DC = D // P   # contraction chunks of 128
CW = 1024     # DMA chunk width (columns)
INV_SQRT_D = float(1.0 / np.sqrt(D))
SQRT_D = float(np.sqrt(D))
EXP_SHIFT = -40.0

F32 = mybir.dt.float32
BF16 = mybir.dt.bfloat16
FP8 = mybir.dt.float8e4
AF = mybir.ActivationFunctionType
ALU = mybir.AluOpType
DR = mybir.MatmulPerfMode.DoubleRow


def build_program(nq, nkv, reps=1):
    nc = bacc.Bacc("TRN2", target_bir_lowering=False, debug=False, num_devices=8)

    NCH = nkv // CW
    NCHh = nq // CW
    # all host-pre-arranged, partition-major, fat contiguous runs
    xA8 = nc.dram_tensor("xA8", [P, NCH * DC * CW], FP8, kind="ExternalInput").ap()
    xB8 = nc.dram_tensor("xB8", [P, NCH * DC * CW], FP8, kind="ExternalInput").ap()
    xAh = nc.dram_tensor("xAh", [P, NCHh * DC * CW], BF16, kind="ExternalInput").ap()
    xBh = nc.dram_tensor("xBh", [P, NCHh * DC * CW], BF16, kind="ExternalInput").ap()
    wq8 = nc.dram_tensor("wq8", [P, DC * D], FP8, kind="ExternalInput").ap()
    wk8 = nc.dram_tensor("wk8", [P, DC * D], FP8, kind="ExternalInput").ap()
    wv8 = nc.dram_tensor("wv8", [P, DC * D], FP8, kind="ExternalInput").ap()
    wcA = nc.dram_tensor("wcA", [P, DC * D], BF16, kind="ExternalInput").ap()
    wcB = nc.dram_tensor("wcB", [P, DC * D], BF16, kind="ExternalInput").ap()
    bq_pc = nc.dram_tensor("bq_pc", [P, DC], F32, kind="ExternalInput").ap()
    bk_pc = nc.dram_tensor("bk_pc", [P, DC], F32, kind="ExternalInput").ap()
    bc_pc = nc.dram_tensor("bc_pc", [P, DC], F32, kind="ExternalInput").ap()
    bv_row = nc.dram_tensor("bv_row", [1, D], BF16, kind="ExternalInput").ap()
    out = nc.dram_tensor("out", [D, nq], F32, kind="ExternalOutput").ap()

    with tile.TileContext(nc) as tc:
        for rep in range(reps):
            _emit_body(
                nc, tc, xA8, xB8, xAh, xBh, wq8, wk8, wv8, wcA, wcB,
                bq_pc, bk_pc, bc_pc, bv_row, out, nq, nkv, rep,
            )

    nc.compile()
    return nc


def _emit_body(
    nc, tc, xA8, xB8, xAh, xBh, wq8, wk8, wv8, wcA, wcB,
    bq_pc, bk_pc, bc_pc, bv_row, out, nq, nkv, rep,
):
    NBQ = nq // P   # query row blocks (16)
    MCK = nkv // P  # key row chunks (32)
    NCH = nkv // CW
    NCHh = nq // CW

    with ExitStack() as st:
        persist = st.enter_context(tc.tile_pool(name="persist", bufs=1))

        # ---------------- phase 1: projections (all local) ----------------
        with ExitStack() as ph1:
            xp = ph1.enter_context(tc.tile_pool(name="xp", bufs=1))
            ps1 = ph1.enter_context(tc.tile_pool(name="ps1", bufs=2, space="PSUM"))

            # critical path on the sync HWDGE queue: K operands, then Q
            wk_sb = persist.tile([P, DC, D], FP8, name="w_wk")
            nc.sync.dma_start(out=wk_sb, in_=wk8.rearrange("p (c o) -> p c o", c=DC))
            xB8_sb = xp.tile([P, NCH, DC, CW], FP8, name="xB8_sb")
            for ci in range(NCH):
                nc.sync.dma_start(
                    out=xB8_sb[:, ci],
                    in_=xB8[:, ci * DC * CW : (ci + 1) * DC * CW].rearrange(
                        "p (c w) -> p c w", c=DC
                    ),
                )
            bk_sb = persist.tile([P, DC], F32, name="bk_sb")
            nc.sync.dma_start(out=bk_sb, in_=bk_pc)
            wq_sb = persist.tile([P, DC, D], FP8, name="w_wq")
            nc.sync.dma_start(out=wq_sb, in_=wq8.rearrange("p (c o) -> p c o", c=DC))
            xA8_sb = xp.tile([P, NCH, DC, CW], FP8, name="xA8_sb")
            for ci in range(NCH):
                nc.sync.dma_start(
                    out=xA8_sb[:, ci],
                    in_=xA8[:, ci * DC * CW : (ci + 1) * DC * CW].rearrange(
                        "p (c w) -> p c w", c=DC
                    ),
                )
            bq_sb = persist.tile([P, DC], F32, name="bq_sb")
            nc.sync.dma_start(out=bq_sb, in_=bq_pc)

            # everything else on the scalar HWDGE queue
            wv_sb = persist.tile([P, DC, D], FP8, name="w_wv")
            nc.scalar.dma_start(out=wv_sb, in_=wv8.rearrange("p (c o) -> p c o", c=DC))
            wcA_sb = persist.tile([P, DC, D], BF16, name="w_wcA")
            nc.scalar.dma_start(out=wcA_sb, in_=wcA.rearrange("p (c o) -> p c o", c=DC))
            wcB_sb = persist.tile([P, DC, D], BF16, name="w_wcB")
            nc.scalar.dma_start(out=wcB_sb, in_=wcB.rearrange("p (c o) -> p c o", c=DC))
            bc_sb = persist.tile([P, DC], F32, name="bc_sb")
            nc.scalar.dma_start(out=bc_sb, in_=bc_pc)
            bv_sb = persist.tile([1, D], BF16, name="bv_sb")
            nc.scalar.dma_start(out=bv_sb, in_=bv_row)
            xAh_sb = xp.tile([P, NCHh, DC, CW], BF16, name="xAh_sb")
            xBh_sb = xp.tile([P, NCHh, DC, CW], BF16, name="xBh_sb")
            for src, dst in ((xAh, xAh_sb), (xBh, xBh_sb)):
                for ci in range(NCHh):
                    nc.scalar.dma_start(
                        out=dst[:, ci],
                        in_=src[:, ci * DC * CW : (ci + 1) * DC * CW].rearrange(
                            "p (c w) -> p c w", c=DC
                        ),
                    )

            ones_row = persist.tile([1, 512], BF16, name="ones_row")
            nc.vector.memset(ones_row, 1.0)
            ones_mat = persist.tile([P, P], BF16, name="ones_mat")
            nc.vector.memset(ones_mat, 1.0)
            shift_sb = persist.tile([P, 1], F32, name="shift_sb")
            nc.vector.memset(shift_sb, EXP_SHIFT)

            qT_sb = persist.tile([P, DC, nq], FP8, name="qT_sb")
            kT_sb = persist.tile([P, DC, nkv], FP8, name="kT_sb")
            v_sb = persist.tile([P, MCK, D], FP8, name="v_sb")
            coT_sb = persist.tile([P, DC, nq], BF16, name="coT_sb")

            # K over all keys (fp8 DoubleRow, N=1024), then Q over own rows
            for s2 in range(0, nkv, 2048):
                for ob in range(DC):
                    ps = ps1.tile([P, 2048], F32, name="ps1t", tag="ps1")
                    for q4 in range(4):
                        s0 = s2 + q4 * 512
                        ci, off = s0 // CW, s0 % CW
                        for c2 in range(DC // 2):
                            nc.tensor.matmul(
                                ps[:, q4 * 512 : (q4 + 1) * 512],
                                lhsT=wk_sb[:, 2 * c2 : 2 * c2 + 2, ob * P : (ob + 1) * P],
                                rhs=xB8_sb[:, ci, 2 * c2 : 2 * c2 + 2, off : off + 512],
                                start=(c2 == 0),
                                stop=(c2 == DC // 2 - 1),
                                perf_mode=DR,
                            )
                    nc.scalar.activation(
                        out=kT_sb[:, ob, s2 : s2 + 2048],
                        in_=ps,
                        func=AF.Identity,
                        bias=bk_sb[:, ob : ob + 1],
                        scale=1.0,
                    )
            for ob in range(DC):
                ps = ps1.tile([P, 2048], F32, name="ps1t", tag="ps1")
                for q4 in range(4):
                    s0 = q4 * 512
                    ci, off = s0 // CW, s0 % CW
                    for c2 in range(DC // 2):
                        nc.tensor.matmul(
                            ps[:, q4 * 512 : (q4 + 1) * 512],
                            lhsT=wq_sb[:, 2 * c2 : 2 * c2 + 2, ob * P : (ob + 1) * P],
                            rhs=xA8_sb[:, ci, 2 * c2 : 2 * c2 + 2, off : off + 512],
                            start=(c2 == 0),
                            stop=(c2 == DC // 2 - 1),
                            perf_mode=DR,
                        )
                nc.scalar.activation(
                    out=qT_sb[:, ob, :],
                    in_=ps,
                    func=AF.Identity,
                    bias=bq_sb[:, ob : ob + 1],
                    scale=1.0,
                )

            # co^T = relu(WcA^T xA + WcB^T xB + bc): transposed (feature on
            # partitions) so bc+relu fuse into the ACT drain per-partition
            for ob in range(DC):
                ps = ps1.tile([P, 2048], F32, name="ps1t", tag="ps1")
                for q4 in range(4):
                    s0 = q4 * 512
                    ci, off = s0 // CW, s0 % CW
                    first = True
                    for w_sb, xh_sb in ((wcA_sb, xAh_sb), (wcB_sb, xBh_sb)):
                        for c in range(DC):
                            last = w_sb is wcB_sb and c == DC - 1
                            nc.tensor.matmul(
                                ps[:, q4 * 512 : (q4 + 1) * 512],
                                lhsT=w_sb[:, c, ob * P : (ob + 1) * P],
                                rhs=xh_sb[:, ci, c, off : off + 512],
                                start=first,
                                stop=last,
                            )
                            first = False
                nc.scalar.activation(
                    out=coT_sb[:, ob, :],
                    in_=ps,
                    func=AF.Relu,
                    bias=bc_sb[:, ob : ob + 1],
                    scale=1.0,
                )

            # V over all key rows of the A view (fp8 DoubleRow), e4m3 out
            for mg in range(0, MCK, 4):
                ps = ps1.tile([P, 2048], F32, name="ps1t", tag="ps1")
                for mi in range(4):
                    m = mg + mi
                    ci, off = (m * P) // CW, (m * P) % CW
                    for c2 in range(DC // 2):
                        nc.tensor.matmul(
                            ps[:, mi * 512 : (mi + 1) * 512],
                            lhsT=xA8_sb[:, ci, 2 * c2 : 2 * c2 + 2, off : off + P],
                            rhs=wv_sb[:, 2 * c2 : 2 * c2 + 2, :],
                            start=(c2 == 0),
                            stop=(c2 == DC // 2 - 1),
                            perf_mode=DR,
                        )
                nc.scalar.activation(out=v_sb[:, mg : mg + 4, :], in_=ps, func=AF.Copy)

        # ---------------- phase 2: attention (S^T layout) ----------------
        at_pool = st.enter_context(tc.tile_pool(name="at_pool", bufs=2))
        a8_pool = st.enter_context(tc.tile_pool(name="a8_pool", bufs=2))
        r_pool = st.enter_context(tc.tile_pool(name="r_pool", bufs=2))
        o_pool = st.enter_context(tc.tile_pool(name="o_pool", bufs=4))
        sps_pool = st.enter_context(tc.tile_pool(name="sps", bufs=2, space="PSUM"))
        sum_pool = st.enter_context(tc.tile_pool(name="sump", bufs=1, space="PSUM"))
        pv_pool = st.enter_context(tc.tile_pool(name="pv", bufs=3, space="PSUM"))

        for s0 in range(0, nq, 512):
            at_sb = at_pool.tile([P, MCK, 512], BF16, name="at_sb", tag="at")
            at8 = a8_pool.tile([P, MCK, 512], FP8, name="at8", tag="at8")
            # rowsum, broadcast across partitions by the all-ones lhsT
            ssum = sum_pool.tile([P, 512], F32, name="ssum", tag="ssum")
            for mbp in range(MCK // 2):
                sps = sps_pool.tile([P, 1024], F32, name="sps", tag="sps")
                for mi in range(2):
                    mb = 2 * mbp + mi
                    for c2 in range(DC // 2):
                        nc.tensor.matmul(
                            sps[:, mi * 512 : (mi + 1) * 512],
                            lhsT=kT_sb[:, 2 * c2 : 2 * c2 + 2, mb * P : (mb + 1) * P],
                            rhs=qT_sb[:, 2 * c2 : 2 * c2 + 2, s0 : s0 + 512],
                            start=(c2 == 0),
                            stop=(c2 == DC // 2 - 1),
                            perf_mode=DR,
                        )
                nc.scalar.activation(
                    out=at_sb[:, 2 * mbp : 2 * mbp + 2, :],
                    in_=sps,
                    func=AF.Exp,
                    bias=shift_sb,
                    scale=1.0,
                )
                for mi in range(2):
                    mb = 2 * mbp + mi
                    nc.tensor.matmul(
                        ssum,
                        lhsT=ones_mat,
                        rhs=at_sb[:, mb, :],
                        start=(mb == 0),
                        stop=(mb == MCK - 1),
                    )
            # 1/rowsum (already broadcast); bf16 so the normalize
            # multiplies run in the DVE 2x packed mode
            rs_bc = r_pool.tile([P, 512], BF16, name="rs_bc", tag="rsb")
            with nc.allow_low_precision(reason="bf16 1/rowsum: 0.4% on a term ~5% of |out|"):
                nc.vector.reciprocal(out=rs_bc, in_=ssum)
            for mb in range(MCK):
                nc.vector.tensor_mul(at8[:, mb, :], at_sb[:, mb, :], rs_bc)

            # PV^T: partitions = output feature; bv enters the PSUM group
            # via ones x bv (rows sum to 1 -> lands as +bv/sqrt(D))
            for j in range(DC):
                pv = pv_pool.tile([P, 512], F32, name="pv", tag="pv")
                nc.tensor.matmul(
                    pv,
                    lhsT=bv_sb[:, j * P : (j + 1) * P],
                    rhs=ones_row,
                    start=True,
                    stop=False,
                )
                for i2 in range(MCK // 2):
                    nc.tensor.matmul(
                        pv,
                        lhsT=v_sb[:, 2 * i2 : 2 * i2 + 2, j * P : (j + 1) * P],
                        rhs=at8[:, 2 * i2 : 2 * i2 + 2, :],
                        start=False,
                        stop=(i2 == MCK // 2 - 1),
                        perf_mode=DR,
                    )
                outt = o_pool.tile([P, 512], F32, name="outt", tag="outt")
                nc.vector.scalar_tensor_tensor(
                    out=outt,
                    in0=pv,
                    scalar=INV_SQRT_D,
                    in1=coT_sb[:, j, s0 : s0 + 512],
                    op0=ALU.mult,
                    op1=ALU.add,
                )
                nc.sync.dma_start(
                    out=out[j * P : (j + 1) * P, s0 : s0 + 512], in_=outt
                )


_PROG_CACHE = {}


def _get_program(nq, nkv):
    key = (nq, nkv)
    if key not in _PROG_CACHE:
        _PROG_CACHE[key] = build_program(nq, nkv)
    return _PROG_CACHE[key]


def _pc_chunks(xT, dt):
    """[D, N] feature-major -> [P, NCH*DC*CW] partition-major fat runs."""
    Dd, N = xT.shape
    nch = N // CW
    return np.ascontiguousarray(
        xT.reshape(DC, P, nch, CW).transpose(1, 2, 0, 3).reshape(P, -1)
    ).astype(dt)


def _pc_weight(WT, dt):
    """[D, D] (in, out) -> [P, DC*D] partition-major."""
    return np.ascontiguousarray(
        WT.reshape(DC, P, D).transpose(1, 0, 2).reshape(P, -1)
    ).astype(dt)


def make_in_maps(x_f, x_s, Wq, bq, Wk, bk, Wv, bv, Wc, bc):
    """Per-core SPMD input dicts + (direction, batch, half) layout."""
    x_f = np.asarray(x_f, np.float32)
    x_s = np.asarray(x_s, np.float32)
    B, N, _ = x_f.shape
    nq = N // 2
    bf = ml_dtypes.bfloat16
    e4 = ml_dtypes.float8_e4m3
    Wq8 = _pc_weight(np.asarray(Wq, np.float32).T, e4)
    Wk8 = _pc_weight(np.asarray(Wk, np.float32).T, e4)
    Wv8 = _pc_weight(np.asarray(Wv, np.float32).T, e4)
    Wc = np.asarray(Wc, np.float32)
    WcfT = _pc_weight(np.ascontiguousarray(Wc[:, :D].T), bf)
    WcsT = _pc_weight(np.ascontiguousarray(Wc[:, D:].T), bf)
    bq32, bk32, bv32, bc32 = (
        np.asarray(b, np.float32) for b in (bq, bk, bv, bc)
    )
    bq_pc = np.ascontiguousarray(bq32.reshape(DC, P).T)
    bk_pc = np.ascontiguousarray(bk32.reshape(DC, P).T)
    bc_pc = np.ascontiguousarray(bc32.reshape(DC, P).T)
    # rows sum to 1 after L1-normalize, so +bv in the PV PSUM lands as
    # +bv/sqrt(D) in the output after the epilogue's 1/sqrt(D) scale --
    # exactly the reference's V-projection bias term
    bv_row = np.ascontiguousarray(bv32[None, :]).astype(bf)
    xT_f = [np.ascontiguousarray(x_f[b].T) for b in range(B)]
    xT_s = [np.ascontiguousarray(x_s[b].T) for b in range(B)]
    in_maps, layout = [], []
    for d in range(2):
        for b in range(B):
            for h in range(2):
                xq = xT_f[b] if d == 0 else xT_s[b]
                xk = xT_s[b] if d == 0 else xT_f[b]
                if h == 1:
                    idx = np.r_[nq:N, 0:nq]
                    xq, xk = xq[:, idx], xk[:, idx]
                in_maps.append(
                    {
                        "xA8": _pc_chunks(xq, e4),
                        "xB8": _pc_chunks(xk, e4),
                        "xAh": _pc_chunks(xq[:, :nq], bf),
                        "xBh": _pc_chunks(xk[:, :nq], bf),
                        "wq8": Wq8,
                        "wk8": Wk8,
                        "wv8": Wv8,
                        "wcA": WcfT if d == 0 else WcsT,
                        "wcB": WcsT if d == 0 else WcfT,
                        "bq_pc": bq_pc,
                        "bk_pc": bk_pc,
                        "bc_pc": bc_pc,
                        "bv_row": bv_row,
                    }
                )
                layout.append((d, b, h))
    return in_maps, layout


def kernel(x_f, x_s, Wq, bq, Wk, bk, Wv, bv, Wc, bc):
    x_f = np.asarray(x_f, np.float32)
    B, N, _ = x_f.shape
    nq = N // 2
    nc = _get_program(nq, N)
    in_maps, layout = make_in_maps(x_f, x_s, Wq, bq, Wk, bk, Wv, bv, Wc, bc)

    from concourse.bass_utils import run_bass_kernel_spmd

    res = run_bass_kernel_spmd(nc, in_maps, list(range(len(in_maps))))
    out_f = np.empty((B, N, D), np.float32)
    out_s = np.empty((B, N, D), np.float32)
    for (d, b, h), r in zip(layout, res.results):
        tgt = out_f if d == 0 else out_s
        tgt[b, h * nq : (h + 1) * nq] = r["out"].T
    return out_f, out_s


# revision 12
# speedup vs baseline: 1.0351x; 1.0085x over previous
"""Cross-view attention (nn_CrossViewAttention) Trainium2 Bass kernel.

Reference computation (B=2, N=4096, D=512):
    co    = relu(concat([x_f, x_s], -1) @ Wc.T + bc)
    out_f = attend(x_f@Wq.T+bq, x_s@Wk.T+bk, x_f@Wv.T+bv) + co
    out_s = attend(x_s@Wq.T+bq, x_f@Wk.T+bk, x_s@Wv.T+bv) + co
    attend(Q,K,V) = (softmax(Q K^T) / L1 / sqrt(D)) @ V

Sharding: 8 cores = (direction f/s) x (batch 0/1) x (sequence half).
Each core computes 2048 output rows of one direction against the full
4096-row K/V for its (direction, batch), SPMD with per-core input data.
Rows are permuted host-side so every core's own rows come first; the
attention reduction over keys is permutation invariant.  Collectives
proved to serialize the whole kernel for a minor PE saving, so K/V/co
are computed fully locally.

Design notes (v3):
  - Q/K/V projections run in fp8-e4m3 DoubleRow (host ships e4m3 copies
    of x^T and the weights); the co MLP stays bf16 (co dominates the
    output magnitude; fp8 there fails the accuracy gate).  bf16 x ships
    only for the core's own 2048 rows (co's input).
  - Every DMA-able tensor is pre-arranged on the host so each partition
    reads one fat contiguous run (4-16 KiB descriptors); the v2 layout
    produced 0.5-1 KiB descriptors that starved the first matmuls.
  - Input DMAs split across the two HWDGE queues (sync carries the K/Q
    operands, scalar everything else) so the first matmuls start right
    after the NEFF preamble.
  - Projections accumulate into [128,2048] PSUM tiles (4 banks, one
    8-bank pool of 2) and drain with FD=2048 ACT ops, so ACT no longer
    paces the PE during phase 1 (v2 lost ~146ns per 2-matmul group).
  - co and PV produce TRANSPOSED tiles (partitions = output feature):
    bc+relu fuse into the co ACT drain as a per-partition bias, bv
    enters the PV PSUM group via a ones-row matmul (rows sum to 1 after
    L1-normalize, so +bv there lands as +bv/sqrt(D) in the output), and
    the epilogue stays one fused DVE op out^T = pv^T/sqrt(D) + co^T.
    The kernel emits out^T [D, nq]; the host transposes for free.
  - Scores run e4m3 DoubleRow into [128,1024] PSUM tiles; exp outputs
    bf16 at FD=1024.  Row sums come from an all-ones [128,128]
    stationary matmul so they land in PSUM already broadcast across
    partitions; one DVE reciprocal yields bf16 1/rowsum, and the
    normalize multiplies run in the DVE 2x bf16 mode, emitting e4m3
    probabilities in [0,1] for the PV DoubleRow matmul.
"""

import sys
from contextlib import ExitStack

for _p in ("/opt/trn_rl_repo", "/root/.axon_site/_ro/trn_rl_repo"):
    if _p not in sys.path:
        sys.path.insert(0, _p)

import ml_dtypes
import numpy as np

import concourse.bacc as bacc
import concourse.bass as bass
import concourse.mybir as mybir
import concourse.tile as tile

P = 128
D = 512
DC = D // P   # contraction chunks of 128
CW = 1024     # DMA chunk width (columns)
INV_SQRT_D = float(1.0 / np.sqrt(D))
SQRT_D = float(np.sqrt(D))
EXP_SHIFT = -40.0

F32 = mybir.dt.float32
BF16 = mybir.dt.bfloat16
FP8 = mybir.dt.float8e4
AF = mybir.ActivationFunctionType
ALU = mybir.AluOpType
DR = mybir.MatmulPerfMode.DoubleRow


def build_program(nq, nkv, reps=1):
    nc = bacc.Bacc("TRN2", target_bir_lowering=False, debug=False, num_devices=8)

    NCH = nkv // CW
    NCHh = nq // CW
    # all host-pre-arranged, partition-major, fat contiguous runs
    xA8 = nc.dram_tensor("xA8", [P, NCH * DC * CW], FP8, kind="ExternalInput").ap()
    xB8 = nc.dram_tensor("xB8", [P, NCH * DC * CW], FP8, kind="ExternalInput").ap()
    xAh = nc.dram_tensor("xAh", [P, NCHh * DC * CW], BF16, kind="ExternalInput").ap()
    xBh = nc.dram_tensor("xBh", [P, NCHh * DC * CW], BF16, kind="ExternalInput").ap()
    wq8 = nc.dram_tensor("wq8", [P, DC * D], FP8, kind="ExternalInput").ap()
    wk8 = nc.dram_tensor("wk8", [P, DC * D], FP8, kind="ExternalInput").ap()
    wv8 = nc.dram_tensor("wv8", [P, DC * D], FP8, kind="ExternalInput").ap()
    wcA = nc.dram_tensor("wcA", [P, DC * D], BF16, kind="ExternalInput").ap()
    wcB = nc.dram_tensor("wcB", [P, DC * D], BF16, kind="ExternalInput").ap()
    bq_pc = nc.dram_tensor("bq_pc", [P, DC], F32, kind="ExternalInput").ap()
    bk_pc = nc.dram_tensor("bk_pc", [P, DC], F32, kind="ExternalInput").ap()
    bc_pc = nc.dram_tensor("bc_pc", [P, DC], F32, kind="ExternalInput").ap()
    bv_row = nc.dram_tensor("bv_row", [1, D], BF16, kind="ExternalInput").ap()
    out = nc.dram_tensor("out", [D, nq], F32, kind="ExternalOutput").ap()

    with tile.TileContext(nc) as tc:
        for rep in range(reps):
            _emit_body(
                nc, tc, xA8, xB8, xAh, xBh, wq8, wk8, wv8, wcA, wcB,
                bq_pc, bk_pc, bc_pc, bv_row, out, nq, nkv, rep,
            )

    nc.compile()
    return nc


def _emit_body(
    nc, tc, xA8, xB8, xAh, xBh, wq8, wk8, wv8, wcA, wcB,
    bq_pc, bk_pc, bc_pc, bv_row, out, nq, nkv, rep,
):
    NBQ = nq // P   # query row blocks (16)
    MCK = nkv // P  # key row chunks (32)
    NCH = nkv // CW
    NCHh = nq // CW

    with ExitStack() as st:
        persist = st.enter_context(tc.tile_pool(name="persist", bufs=1))

        # ---------------- phase 1: projections (all local) ----------------
        with ExitStack() as ph1:
            xp = ph1.enter_context(tc.tile_pool(name="xp", bufs=1))
            ps1 = ph1.enter_context(tc.tile_pool(name="ps1", bufs=2, space="PSUM"))

            # x8/xh chunks alternate between the two HWDGE queues (sync +
            # scalar): each queue executes its DMAs serially incl. a ~2us
            # completion latency, so one-queue delivery starved the first
            # matmuls for ~11us.
            def _chunk(dram, ci):
                return dram[:, ci * DC * CW : (ci + 1) * DC * CW].rearrange(
                    "p (c w) -> p c w", c=DC
                )

            wk_sb = persist.tile([P, DC, D], FP8, name="w_wk")
            nc.sync.dma_start(out=wk_sb, in_=wk8.rearrange("p (c o) -> p c o", c=DC))
            wq_sb = persist.tile([P, DC, D], FP8, name="w_wq")
            nc.scalar.dma_start(out=wq_sb, in_=wq8.rearrange("p (c o) -> p c o", c=DC))
            xB8_sb = xp.tile([P, NCH, DC, CW], FP8, name="xB8_sb")
            xA8_sb = xp.tile([P, NCH, DC, CW], FP8, name="xA8_sb")
            for ci in range(NCH):
                eng = nc.sync if ci % 2 == 0 else nc.scalar
                eng.dma_start(out=xB8_sb[:, ci], in_=_chunk(xB8, ci))
            bk_sb = persist.tile([P, DC], F32, name="bk_sb")
            nc.sync.dma_start(out=bk_sb, in_=bk_pc)
            wv_sb = persist.tile([P, DC, D], FP8, name="w_wv")
            nc.scalar.dma_start(out=wv_sb, in_=wv8.rearrange("p (c o) -> p c o", c=DC))
            for ci in range(NCH):
                eng = nc.sync if ci % 2 == 1 else nc.scalar
                eng.dma_start(out=xA8_sb[:, ci], in_=_chunk(xA8, ci))
            bq_sb = persist.tile([P, DC], F32, name="bq_sb")
            nc.sync.dma_start(out=bq_sb, in_=bq_pc)

            wcA_sb = persist.tile([P, DC, D], BF16, name="w_wcA")
            nc.scalar.dma_start(out=wcA_sb, in_=wcA.rearrange("p (c o) -> p c o", c=DC))
            wcB_sb = persist.tile([P, DC, D], BF16, name="w_wcB")
            nc.sync.dma_start(out=wcB_sb, in_=wcB.rearrange("p (c o) -> p c o", c=DC))
            bc_sb = persist.tile([P, DC], F32, name="bc_sb")
            nc.scalar.dma_start(out=bc_sb, in_=bc_pc)
            bv_sb = persist.tile([1, D], BF16, name="bv_sb")
            nc.scalar.dma_start(out=bv_sb, in_=bv_row)
            xAh_sb = xp.tile([P, NCHh, DC, CW], BF16, name="xAh_sb")
            xBh_sb = xp.tile([P, NCHh, DC, CW], BF16, name="xBh_sb")
            for ci in range(NCHh):
                nc.sync.dma_start(out=xAh_sb[:, ci], in_=_chunk(xAh, ci))
                nc.scalar.dma_start(out=xBh_sb[:, ci], in_=_chunk(xBh, ci))

            ones_row = persist.tile([1, 512], BF16, name="ones_row")
            nc.vector.memset(ones_row, 1.0)
            ones_mat = persist.tile([P, P], BF16, name="ones_mat")
            nc.vector.memset(ones_mat, 1.0)
            shift_sb = persist.tile([P, 1], F32, name="shift_sb")
            nc.vector.memset(shift_sb, EXP_SHIFT)

            # warm-up while the first DMAs land: junk matmuls hold the PE
            # HAM clock-gate at 2.4 GHz for the real work, and a 1-column
            # exp preloads the ACT table set (~2.7us otherwise paid right
            # before the first score tile).
            warm_act = persist.tile([P, 1], F32, name="warm_act")
            nc.scalar.activation(out=warm_act, in_=shift_sb, func=AF.Exp)
            warm_ps = ps1.tile([P, 2048], F32, name="ps1t", tag="ps1")
            for _wi in range(8):
                nc.tensor.matmul(
                    warm_ps[:, :512],
                    lhsT=ones_row[:, :P],
                    rhs=ones_row,
                    start=True,
                    stop=True,
                )

            qT_sb = persist.tile([P, DC, nq], FP8, name="qT_sb")
            kT_sb = persist.tile([P, DC, nkv], FP8, name="kT_sb")
            v_sb = persist.tile([P, MCK, D], FP8, name="v_sb")
            coT_sb = persist.tile([P, DC, nq], BF16, name="coT_sb")

            # K over all keys (fp8 DoubleRow, N=1024), then Q over own rows
            for s2 in range(0, nkv, 2048):
                for ob in range(DC):
                    ps = ps1.tile([P, 2048], F32, name="ps1t", tag="ps1")
                    for q4 in range(4):
                        s0 = s2 + q4 * 512
                        ci, off = s0 // CW, s0 % CW
                        for c2 in range(DC // 2):
                            nc.tensor.matmul(
                                ps[:, q4 * 512 : (q4 + 1) * 512],
                                lhsT=wk_sb[:, 2 * c2 : 2 * c2 + 2, ob * P : (ob + 1) * P],
                                rhs=xB8_sb[:, ci, 2 * c2 : 2 * c2 + 2, off : off + 512],
                                start=(c2 == 0),
                                stop=(c2 == DC // 2 - 1),
                                perf_mode=DR,
                            )
                    nc.scalar.activation(
                        out=kT_sb[:, ob, s2 : s2 + 2048],
                        in_=ps,
                        func=AF.Identity,
                        bias=bk_sb[:, ob : ob + 1],
                        scale=1.0,
                    )
            for ob in range(DC):
                ps = ps1.tile([P, 2048], F32, name="ps1t", tag="ps1")
                for q4 in range(4):
                    s0 = q4 * 512
                    ci, off = s0 // CW, s0 % CW
                    for c2 in range(DC // 2):
                        nc.tensor.matmul(
                            ps[:, q4 * 512 : (q4 + 1) * 512],
                            lhsT=wq_sb[:, 2 * c2 : 2 * c2 + 2, ob * P : (ob + 1) * P],
                            rhs=xA8_sb[:, ci, 2 * c2 : 2 * c2 + 2, off : off + 512],
                            start=(c2 == 0),
                            stop=(c2 == DC // 2 - 1),
                            perf_mode=DR,
                        )
                nc.scalar.activation(
                    out=qT_sb[:, ob, :],
                    in_=ps,
                    func=AF.Identity,
                    bias=bq_sb[:, ob : ob + 1],
                    scale=1.0,
                )

            # V over all key rows of the A view (fp8 DoubleRow), e4m3 out;
            # emitted before co so the PE has work while co's bf16 x
            # halves are still streaming in
            for mg in range(0, MCK, 4):
                ps = ps1.tile([P, 2048], F32, name="ps1t", tag="ps1")
                for mi in range(4):
                    m = mg + mi
                    ci, off = (m * P) // CW, (m * P) % CW
                    for c2 in range(DC // 2):
                        nc.tensor.matmul(
                            ps[:, mi * 512 : (mi + 1) * 512],
                            lhsT=xA8_sb[:, ci, 2 * c2 : 2 * c2 + 2, off : off + P],
                            rhs=wv_sb[:, 2 * c2 : 2 * c2 + 2, :],
                            start=(c2 == 0),
                            stop=(c2 == DC // 2 - 1),
                            perf_mode=DR,
                        )
                nc.scalar.activation(out=v_sb[:, mg : mg + 4, :], in_=ps, func=AF.Copy)

            # co^T = relu(WcA^T xA + WcB^T xB + bc): transposed (feature on
            # partitions) so bc+relu fuse into the ACT drain per-partition
            for ob in range(DC):
                ps = ps1.tile([P, 2048], F32, name="ps1t", tag="ps1")
                for q4 in range(4):
                    s0 = q4 * 512
                    ci, off = s0 // CW, s0 % CW
                    first = True
                    for w_sb, xh_sb in ((wcA_sb, xAh_sb), (wcB_sb, xBh_sb)):
                        for c in range(DC):
                            last = w_sb is wcB_sb and c == DC - 1
                            nc.tensor.matmul(
                                ps[:, q4 * 512 : (q4 + 1) * 512],
                                lhsT=w_sb[:, c, ob * P : (ob + 1) * P],
                                rhs=xh_sb[:, ci, c, off : off + 512],
                                start=first,
                                stop=last,
                            )
                            first = False
                nc.scalar.activation(
                    out=coT_sb[:, ob, :],
                    in_=ps,
                    func=AF.Relu,
                    bias=bc_sb[:, ob : ob + 1],
                    scale=1.0,
                )

        # ---------------- phase 2: attention (S^T layout) ----------------
        at_pool = st.enter_context(tc.tile_pool(name="at_pool", bufs=2))
        a8_pool = st.enter_context(tc.tile_pool(name="a8_pool", bufs=2))
        r_pool = st.enter_context(tc.tile_pool(name="r_pool", bufs=2))
        o_pool = st.enter_context(tc.tile_pool(name="o_pool", bufs=4))
        sps_pool = st.enter_context(tc.tile_pool(name="sps", bufs=2, space="PSUM"))
        sum_pool = st.enter_context(tc.tile_pool(name="sump", bufs=1, space="PSUM"))
        pv_pool = st.enter_context(tc.tile_pool(name="pv", bufs=3, space="PSUM"))

        for s0 in range(0, nq, 512):
            at_sb = at_pool.tile([P, MCK, 512], BF16, name="at_sb", tag="at")
            at8 = a8_pool.tile([P, MCK, 512], FP8, name="at8", tag="at8")
            # rowsum, broadcast across partitions by the all-ones lhsT
            ssum = sum_pool.tile([P, 512], F32, name="ssum", tag="ssum")
            for mbp in range(MCK // 2):
                sps = sps_pool.tile([P, 1024], F32, name="sps", tag="sps")
                for mi in range(2):
                    mb = 2 * mbp + mi
                    for c2 in range(DC // 2):
                        nc.tensor.matmul(
                            sps[:, mi * 512 : (mi + 1) * 512],
                            lhsT=kT_sb[:, 2 * c2 : 2 * c2 + 2, mb * P : (mb + 1) * P],
                            rhs=qT_sb[:, 2 * c2 : 2 * c2 + 2, s0 : s0 + 512],
                            start=(c2 == 0),
                            stop=(c2 == DC // 2 - 1),
                            perf_mode=DR,
                        )
                nc.scalar.activation(
                    out=at_sb[:, 2 * mbp : 2 * mbp + 2, :],
                    in_=sps,
                    func=AF.Exp,
                    bias=shift_sb,
                    scale=1.0,
                )
                for mi in range(2):
                    mb = 2 * mbp + mi
                    nc.tensor.matmul(
                        ssum,
                        lhsT=ones_mat,
                        rhs=at_sb[:, mb, :],
                        start=(mb == 0),
                        stop=(mb == MCK - 1),
                    )
            # 1/rowsum (already broadcast); bf16 so the normalize
            # multiplies run in the DVE 2x packed mode
            rs_bc = r_pool.tile([P, 512], BF16, name="rs_bc", tag="rsb")
            with nc.allow_low_precision(reason="bf16 1/rowsum: 0.4% on a term ~5% of |out|"):
                nc.vector.reciprocal(out=rs_bc, in_=ssum)
            for mb in range(MCK):
                nc.vector.tensor_mul(at8[:, mb, :], at_sb[:, mb, :], rs_bc)

            # PV^T: partitions = output feature; bv enters the PSUM group
            # via ones x bv (rows sum to 1 -> lands as +bv/sqrt(D))
            for j in range(DC):
                pv = pv_pool.tile([P, 512], F32, name="pv", tag="pv")
                nc.tensor.matmul(
                    pv,
                    lhsT=bv_sb[:, j * P : (j + 1) * P],
                    rhs=ones_row,
                    start=True,
                    stop=False,
                )
                for i2 in range(MCK // 2):
                    nc.tensor.matmul(
                        pv,
                        lhsT=v_sb[:, 2 * i2 : 2 * i2 + 2, j * P : (j + 1) * P],
                        rhs=at8[:, 2 * i2 : 2 * i2 + 2, :],
                        start=False,
                        stop=(i2 == MCK // 2 - 1),
                        perf_mode=DR,
                    )
                outt = o_pool.tile([P, 512], F32, name="outt", tag="outt")
                nc.vector.scalar_tensor_tensor(
                    out=outt,
                    in0=pv,
                    scalar=INV_SQRT_D,
                    in1=coT_sb[:, j, s0 : s0 + 512],
                    op0=ALU.mult,
                    op1=ALU.add,
                )
                nc.sync.dma_start(
                    out=out[j * P : (j + 1) * P, s0 : s0 + 512], in_=outt
                )


_PROG_CACHE = {}


def _get_program(nq, nkv):
    key = (nq, nkv)
    if key not in _PROG_CACHE:
        _PROG_CACHE[key] = build_program(nq, nkv)
    return _PROG_CACHE[key]


def _pc_chunks(xT, dt):
    """[D, N] feature-major -> [P, NCH*DC*CW] partition-major fat runs."""
    Dd, N = xT.shape
    nch = N // CW
    return np.ascontiguousarray(
        xT.reshape(DC, P, nch, CW).transpose(1, 2, 0, 3).reshape(P, -1)
    ).astype(dt)


def _pc_weight(WT, dt):
    """[D, D] (in, out) -> [P, DC*D] partition-major."""
    return np.ascontiguousarray(
        WT.reshape(DC, P, D).transpose(1, 0, 2).reshape(P, -1)
    ).astype(dt)


def make_in_maps(x_f, x_s, Wq, bq, Wk, bk, Wv, bv, Wc, bc):
    """Per-core SPMD input dicts + (direction, batch, half) layout."""
    x_f = np.asarray(x_f, np.float32)
    x_s = np.asarray(x_s, np.float32)
    B, N, _ = x_f.shape
    nq = N // 2
    bf = ml_dtypes.bfloat16
    e4 = ml_dtypes.float8_e4m3
    Wq8 = _pc_weight(np.asarray(Wq, np.float32).T, e4)
    Wk8 = _pc_weight(np.asarray(Wk, np.float32).T, e4)
    Wv8 = _pc_weight(np.asarray(Wv, np.float32).T, e4)
    Wc = np.asarray(Wc, np.float32)
    WcfT = _pc_weight(np.ascontiguousarray(Wc[:, :D].T), bf)
    WcsT = _pc_weight(np.ascontiguousarray(Wc[:, D:].T), bf)
    bq32, bk32, bv32, bc32 = (
        np.asarray(b, np.float32) for b in (bq, bk, bv, bc)
    )
    bq_pc = np.ascontiguousarray(bq32.reshape(DC, P).T)
    bk_pc = np.ascontiguousarray(bk32.reshape(DC, P).T)
    bc_pc = np.ascontiguousarray(bc32.reshape(DC, P).T)
    # rows sum to 1 after L1-normalize, so +bv in the PV PSUM lands as
    # +bv/sqrt(D) in the output after the epilogue's 1/sqrt(D) scale --
    # exactly the reference's V-projection bias term
    bv_row = np.ascontiguousarray(bv32[None, :]).astype(bf)
    xT_f = [np.ascontiguousarray(x_f[b].T) for b in range(B)]
    xT_s = [np.ascontiguousarray(x_s[b].T) for b in range(B)]
    in_maps, layout = [], []
    for d in range(2):
        for b in range(B):
            for h in range(2):
                xq = xT_f[b] if d == 0 else xT_s[b]
                xk = xT_s[b] if d == 0 else xT_f[b]
                if h == 1:
                    idx = np.r_[nq:N, 0:nq]
                    xq, xk = xq[:, idx], xk[:, idx]
                in_maps.append(
                    {
                        "xA8": _pc_chunks(xq, e4),
                        "xB8": _pc_chunks(xk, e4),
                        "xAh": _pc_chunks(xq[:, :nq], bf),
                        "xBh": _pc_chunks(xk[:, :nq], bf),
                        "wq8": Wq8,
                        "wk8": Wk8,
                        "wv8": Wv8,
                        "wcA": WcfT if d == 0 else WcsT,
                        "wcB": WcsT if d == 0 else WcfT,
                        "bq_pc": bq_pc,
                        "bk_pc": bk_pc,
                        "bc_pc": bc_pc,
                        "bv_row": bv_row,
                    }
                )
                layout.append((d, b, h))
    return in_maps, layout


def kernel(x_f, x_s, Wq, bq, Wk, bk, Wv, bv, Wc, bc):
    x_f = np.asarray(x_f, np.float32)
    B, N, _ = x_f.shape
    nq = N // 2
    nc = _get_program(nq, N)
    in_maps, layout = make_in_maps(x_f, x_s, Wq, bq, Wk, bk, Wv, bv, Wc, bc)

    from concourse.bass_utils import run_bass_kernel_spmd

    res = run_bass_kernel_spmd(nc, in_maps, list(range(len(in_maps))))
    out_f = np.empty((B, N, D), np.float32)
    out_s = np.empty((B, N, D), np.float32)
    for (d, b, h), r in zip(layout, res.results):
        tgt = out_f if d == 0 else out_s
        tgt[b, h * nq : (h + 1) * nq] = r["out"].T
    return out_f, out_s
